# revision 4
# baseline (speedup 1.0000x reference)
"""BigBird GNN message passing on Trainium2, 8-core SPMD.

Sharding: dst-range sharding (SH dsts per core). Per layer:
  1. Each core computes Q (own dsts, +zero row) and K|V (own node shard)
     from its SBUF-resident h^T slice; K|V packed as KV [SH, 256].
  2. AllGather KV halves -> KV_A/KV_B [NC*HALF, 256] (full graph KV,
     int16-gatherable row spaces < 32768).
  3. Edge phase (2 passes A/B): batched dma_gather of KV rows (by src) and
     Q rows (by dst; uniform/dummy edges hit the zero row so exp(0)=1),
     per 128-edge tile: alpha = rowdot(Qg, Kg)*scale -> exp -> Vg*exp,
     one-hot S = (iota == dstloc), matmul-accumulate S^T-weighted sums
     into per-window PSUM -> SBUF accumulators acc_v [D x dst], acc_s.
  4. Finalize per 127-dst window: r = 1/s, out^T = acc_v * r,
     h'^T = Wo^T @ out^T + bo (+relu), into the next h^T slice.

All-masked dsts are pre-detected on host: their full edge list is included
with Q-index = zero row, reproducing the reference's uniform-average
degeneracy exactly. Masked edges of other dsts contribute exp(-1e9-m)=0
in the reference and are simply dropped here.
"""
import math
from dataclasses import dataclass, field

import numpy as np


@dataclass
class Cfg:
    N: int = 40000
    E: int = 640000
    D: int = 128
    L: int = 3
    BLOCK: int = 64
    KEEP_P: float = 0.1
    NC: int = 8
    BT: int = 16          # tiles per gather batch
    TILE: int = 128
    WD: int = 127         # dsts per window (col 127 = trash)

    @property
    def SH(self):
        return self.N // self.NC

    @property
    def HALF(self):
        return self.SH // 2

    @property
    def W(self):
        return (self.SH + self.WD - 1) // self.WD

    @property
    def QROWS(self):
        return self.SH + 1

    @property
    def SCALE(self):
        return 1.0 / math.sqrt(self.D)


def compute_masks(cfg):
    """Reference's per-layer random keep masks, bit-exact (CPU jax)."""
    import jax
    out = []
    with jax.default_device(jax.devices('cpu')[0]):
        for i in range(cfg.L):
            rkey = jax.random.fold_in(jax.random.key(42), i)
            out.append(np.asarray(jax.random.uniform(rkey, (cfg.E,)) > cfg.KEEP_P))
    return out


def prep_edges(cfg, edge_index, rand_masks):
    """Per-layer edge metadata. Returns (counts [L,2,W], meta per core, Ttot)."""
    c_ = cfg
    src = np.asarray(edge_index[0]).astype(np.int64)
    dst = np.asarray(edge_index[1]).astype(np.int64)
    local_keep = np.abs(src - dst) <= c_.BLOCK

    counts = np.zeros((c_.L, 2, c_.W), np.int64)
    per = [[[None] * c_.W for _ in range(2 * c_.NC)] for _ in range(c_.L)]

    for l in range(c_.L):
        kept = local_keep | ~rand_masks[l]
        deg_all = np.bincount(dst, minlength=c_.N)
        kept_cnt = np.bincount(dst[kept], minlength=c_.N)
        uniform_dst = (kept_cnt == 0) & (deg_all > 0)
        use = kept | uniform_dst[dst]
        es, ed = src[use], dst[use]
        is_unif = uniform_dst[ed]
        core = ed // c_.SH
        dloc = ed - core * c_.SH
        win = dloc // c_.WD
        srcmod = es % c_.SH
        half = (srcmod >= c_.HALF).astype(np.int64)
        kvrow = c_.HALF * (es // c_.SH) + srcmod - half * c_.HALF
        qrow = np.where(is_unif, c_.SH, dloc)
        dstcol = dloc - win * c_.WD

        order = np.lexsort((es, win, half, core))
        c_o, h_o, w_o = core[order], half[order], win[order]
        kv_o, q_o, dc_o = kvrow[order], qrow[order], dstcol[order]
        key = (c_o * 2 + h_o) * c_.W + w_o
        bounds = np.searchsorted(key, np.arange(c_.NC * 2 * c_.W + 1))
        for c in range(c_.NC):
            for h in range(2):
                for w in range(c_.W):
                    k = (c * 2 + h) * c_.W + w
                    s, e = bounds[k], bounds[k + 1]
                    per[l][c * 2 + h][w] = (kv_o[s:e], q_o[s:e], dc_o[s:e])
                    counts[l, h, w] = max(counts[l, h, w], -(-(e - s) // c_.TILE))
        counts[l, 0, :] = np.maximum(counts[l, 0, :], 1)

    Ttot = int(counts.sum())
    meta = []
    for c in range(c_.NC):
        kv_all = np.zeros((Ttot, c_.TILE), np.int16)
        q_all = np.full((Ttot, c_.TILE), c_.SH, np.int16)
        dl_all = np.full((Ttot, c_.TILE), c_.WD, np.float32)
        t0 = 0
        for l in range(c_.L):
            for h in range(2):
                for w in range(c_.W):
                    kv, q, dc = per[l][c * 2 + h][w]
                    n = len(kv)
                    nt = int(counts[l, h, w])
                    kv_all[t0:t0 + nt].reshape(-1)[:n] = kv
                    q_all[t0:t0 + nt].reshape(-1)[:n] = q
                    dl_all[t0:t0 + nt].reshape(-1)[:n] = dc
                    t0 += nt
        assert t0 == Ttot

        def wrap16(a):
            b = a.reshape(-1).reshape(-1, 16).T  # [16, T*8]
            return np.ascontiguousarray(np.tile(b, (8, 1)).astype(np.int16))
        meta.append({
            'kvix': wrap16(kv_all),
            'qix': wrap16(q_all),
            'dloc': np.ascontiguousarray(dl_all.T),
        })
    return counts, meta, Ttot


def pack_weights(cfg, inp):
    L = cfg.L
    ws = np.zeros((128, L * 4 * 128), np.float32)
    brow = np.zeros((128, L * 3 * 128), np.float32)  # row-bias replicated over partitions
    boc = np.zeros((128, L), np.float32)
    for l in range(L):
        for f, nm in enumerate(['Wq', 'Wk', 'Wv', 'Wo']):
            ws[:, (l * 4 + f) * 128:(l * 4 + f + 1) * 128] = np.asarray(inp[nm][l], np.float32)
        for f, nm in enumerate(['bq', 'bk', 'bv']):
            brow[:, (l * 3 + f) * 128:(l * 3 + f + 1) * 128] = np.asarray(inp[nm][l], np.float32)[None, :]
        boc[:, l] = np.asarray(inp['bo'][l], np.float32)
    return ws, brow, boc


def build_in_maps(cfg, inputs, counts, meta):
    x = np.asarray(inputs['x'], np.float32)
    ws, brow, boc = pack_weights(cfg, inputs)
    iota = np.broadcast_to(np.arange(128, dtype=np.float32)[None, :], (128, 128)).copy()
    in_maps = []
    for c in range(cfg.NC):
        xT = np.ascontiguousarray(x[c * cfg.SH:(c + 1) * cfg.SH].T)
        in_maps.append({
            'xT': xT, 'ws': ws, 'brow': brow, 'boc': boc, 'iota': iota,
            'kvix': meta[c]['kvix'], 'qix': meta[c]['qix'], 'dloc': meta[c]['dloc'],
        })
    return in_maps


def build_bass(cfg, counts, skip=()):
    import os
    import concourse.bacc as bacc
    import concourse.mybir as mybir
    import concourse.tile as tile
    from concourse import library_config

    c_ = cfg
    f32 = mybir.dt.float32
    i16 = mybir.dt.int16
    AL = mybir.AluOpType
    AF = mybir.ActivationFunctionType
    Ttot = int(counts.sum())
    SH, HALF, W, WD, BT, L, D = c_.SH, c_.HALF, c_.W, c_.WD, c_.BT, c_.L, c_.D
    NCH = -(-SH // 128)           # node chunks for QKV phase

    nc = bacc.Bacc("TRN2", num_devices=c_.NC)

    xT = nc.dram_tensor("xT", [128, SH], f32, kind="ExternalInput")
    ws = nc.dram_tensor("ws", [128, L * 4 * 128], f32, kind="ExternalInput")
    brow = nc.dram_tensor("brow", [128, L * 3 * 128], f32, kind="ExternalInput")
    boc = nc.dram_tensor("boc", [128, L], f32, kind="ExternalInput")
    iota = nc.dram_tensor("iota", [128, 128], f32, kind="ExternalInput")
    kvix = nc.dram_tensor("kvix", [128, Ttot * 8], i16, kind="ExternalInput")
    qix = nc.dram_tensor("qix", [128, Ttot * 8], i16, kind="ExternalInput")
    dloc = nc.dram_tensor("dloc", [128, Ttot], f32, kind="ExternalInput")
    outT = nc.dram_tensor("outT", [128, SH], f32, kind="ExternalOutput")

    Qd = nc.dram_tensor("Qd", [c_.QROWS, 128], f32)
    KVown = nc.dram_tensor("KVown", [SH, 256], f32)
    kv_space = "Local" if 'coll' in skip else "Shared"
    KV_A = nc.dram_tensor("KV_A", [c_.NC * HALF, 256], f32, addr_space=kv_space)
    KV_B = nc.dram_tensor("KV_B", [c_.NC * HALF, 256], f32, addr_space=kv_space)
    groups = [list(range(c_.NC))]

    with tile.TileContext(nc) as tc:
        with (
            tc.tile_pool(name="const", bufs=1) as cpool,
            tc.tile_pool(name="hbuf", bufs=1) as hpool,
            tc.tile_pool(name="acc", bufs=1) as apool,
            tc.tile_pool(name="qkv", bufs=3) as qkvpool,
            tc.tile_pool(name="gath", bufs=2) as gpool,
            tc.tile_pool(name="scr", bufs=4) as spool,
            tc.tile_pool(name="psA", bufs=2, space="PSUM") as psA,
            tc.tile_pool(name="psB", bufs=2, space="PSUM") as psB,
        ):
            nc.gpsimd.load_library(library_config.mlp)
            nidx_regs = {}  # num_idxs value -> Pool register (reused)

            def nidx_reg(v):
                if v not in nidx_regs:
                    nidx_regs[v] = nc.gpsimd.to_reg(v)
                return nidx_regs[v]
            iota_sb = cpool.tile([128, 128], f32, tag="iota")
            ws_sb = cpool.tile([128, L * 4 * 128], f32, tag="ws")
            brow_sb = cpool.tile([128, L * 3 * 128], f32, tag="brow")
            ones_row = cpool.tile([1, 128], f32, tag="ones")
            boc_sb = cpool.tile([128, L], f32, tag="boc")
            zrow = cpool.tile([1, 128], f32, tag="zrow")
            nc.sync.dma_start(out=iota_sb[:], in_=iota[:, :])
            nc.sync.dma_start(out=ws_sb[:], in_=ws[:, :])
            nc.sync.dma_start(out=brow_sb[:], in_=brow[:, :])
            nc.sync.dma_start(out=boc_sb[:], in_=boc[:, :])
            nc.vector.memset(zrow[:], 0.0)
            nc.vector.memset(ones_row[:], 1.0)
            nc.sync.dma_start(out=Qd[SH:SH + 1, :], in_=zrow[:])

            hA = hpool.tile([128, SH], f32, tag="hA")
            hB = hpool.tile([128, SH], f32, tag="hB")
            acc_v = apool.tile([128, W * 128], f32, tag="accv")
            acc_s = apool.tile([1, W * 128], f32, tag="accs")
            nc.sync.dma_start(out=hA[:], in_=xT[:, :])

            hcur, hnext = hA, hB
            t_base = 0
            for l in range(L):
                wq = ws_sb[:, (l * 4 + 0) * 128:(l * 4 + 1) * 128]
                wk = ws_sb[:, (l * 4 + 1) * 128:(l * 4 + 2) * 128]
                wv = ws_sb[:, (l * 4 + 2) * 128:(l * 4 + 3) * 128]
                wo = ws_sb[:, (l * 4 + 3) * 128:(l * 4 + 4) * 128]
                bq = brow_sb[:, (l * 3 + 0) * 128:(l * 3 + 1) * 128]
                bk = brow_sb[:, (l * 3 + 1) * 128:(l * 3 + 2) * 128]
                bv = brow_sb[:, (l * 3 + 2) * 128:(l * 3 + 3) * 128]

                # --- Phase 1: Q / KV for own shard ---
                for ch in range(NCH):
                    cw = min(128, SH - ch * 128)
                    hs = hcur[:, ch * 128:ch * 128 + cw]
                    pq = psA.tile([128, 128], f32, tag="pq")
                    pkv = psA.tile([128, 256], f32, tag="pkv")
                    nc.tensor.matmul(out=pq[:cw, :], lhsT=hs, rhs=wq, start=True, stop=True)
                    nc.tensor.matmul(out=pkv[:cw, 0:128], lhsT=hs, rhs=wk, start=True, stop=True)
                    nc.tensor.matmul(out=pkv[:cw, 128:256], lhsT=hs, rhs=wv, start=True, stop=True)
                    qt = qkvpool.tile([128, 128], f32, tag="qt")
                    kvt = qkvpool.tile([128, 256], f32, tag="kvt")
                    nc.vector.tensor_tensor(
                        out=qt[:cw, :], in0=pq[:cw, :],
                        in1=bq[:cw, :], op=AL.add)
                    nc.vector.tensor_tensor(
                        out=kvt[:cw, 0:128], in0=pkv[:cw, 0:128],
                        in1=bk[:cw, :], op=AL.add)
                    nc.vector.tensor_tensor(
                        out=kvt[:cw, 128:256], in0=pkv[:cw, 128:256],
                        in1=bv[:cw, :], op=AL.add)
                    nc.sync.dma_start(out=Qd[ch * 128:ch * 128 + cw, :], in_=qt[:cw, :])
                    nc.sync.dma_start(out=KVown[ch * 128:ch * 128 + cw, :], in_=kvt[:cw, :])

                # --- Phase 2: AllGather KV halves ---
                if 'coll' not in skip:
                  nc.gpsimd.collective_compute(
                    "AllGather", AL.bypass, replica_groups=groups,
                    ins=[KVown[0:HALF, :]], outs=[KV_A[:, :]])
                  nc.gpsimd.collective_compute(
                    "AllGather", AL.bypass, replica_groups=groups,
                    ins=[KVown[HALF:SH, :]], outs=[KV_B[:, :]])

                # --- Phase 3: edge passes ---
                for h in range(2 if 'edge' not in skip else 0):
                    kvsrc = KV_A if h == 0 else KV_B
                    # tile schedule: (window, first, last) per tile of this pass
                    sched = []
                    for w in range(W):
                        nt = int(counts[l, h, w])
                        for t in range(nt):
                            sched.append((w, t == 0, t == nt - 1))
                    Tp = len(sched)
                    cur_v, cur_s = None, None
                    for b0 in range(0, Tp, BT):
                        bt = min(BT, Tp - b0)
                        g0 = t_base + b0
                        kvi_sb = gpool.tile([128, BT * 8], i16, tag="kvi")
                        qi_sb = gpool.tile([128, BT * 8], i16, tag="qi")
                        dl_sb = gpool.tile([128, BT], f32, tag="dl")
                        nc.sync.dma_start(out=kvi_sb[:, :bt * 8], in_=kvix[:, g0 * 8:(g0 + bt) * 8])
                        nc.sync.dma_start(out=qi_sb[:, :bt * 8], in_=qix[:, g0 * 8:(g0 + bt) * 8])
                        nc.sync.dma_start(out=dl_sb[:, :bt], in_=dloc[:, g0:g0 + bt])
                        kvg = gpool.tile([128, BT, 256], f32, tag="kvg")
                        qg = gpool.tile([128, BT, 128], f32, tag="qg")
                        nc.gpsimd.dma_gather(
                            out_ap=kvg[:, :bt, :], in_ap=kvsrc[:, :],
                            idxs_ap=kvi_sb[:, :bt * 8],
                            num_idxs=bt * 128, num_idxs_reg=nidx_reg(bt * 128),
                            elem_size=256, single_packet=False)
                        nc.gpsimd.dma_gather(
                            out_ap=qg[:, :bt, :], in_ap=Qd[:, :],
                            idxs_ap=qi_sb[:, :bt * 8],
                            num_idxs=bt * 128, num_idxs_reg=nidx_reg(bt * 128),
                            elem_size=128, single_packet=False)
                        prod = gpool.tile([128, BT, 128], f32, tag="prod")
                        nc.vector.tensor_tensor(
                            out=prod[:, :bt, :], in0=qg[:, :bt, :],
                            in1=kvg[:, :bt, 0:128], op=AL.mult)
                        alpha = spool.tile([128, BT], f32, tag="alpha")
                        nc.vector.tensor_reduce(
                            out=alpha[:, :bt, None], in_=prod[:, :bt, :],
                            axis=mybir.AxisListType.X, op=AL.add)
                        expf = spool.tile([128, BT], f32, tag="expf")
                        nc.scalar.activation(expf[:, :bt], alpha[:, :bt], AF.Exp,
                                             scale=float(c_.SCALE))
                        vex = gpool.tile([128, BT, 128], f32, tag="vex")
                        nc.vector.tensor_tensor(
                            out=vex[:, :bt, :], in0=kvg[:, :bt, 128:256],
                            in1=expf[:, :bt, None].to_broadcast([128, bt, 128]),
                            op=AL.mult)
                        for t in range(bt):
                            w, first, last = sched[b0 + t]
                            S = spool.tile([128, 128], f32, tag="S")
                            nc.vector.tensor_scalar(
                                out=S[:], in0=iota_sb[:],
                                scalar1=dl_sb[:, t:t + 1], scalar2=None, op0=AL.is_equal)
                            if first:
                                cur_v = psB.tile([128, 128], f32, tag="pv")
                                cur_s = psB.tile([1, 128], f32, tag="ps")
                            nc.tensor.matmul(out=cur_v[:], lhsT=vex[:, t, :], rhs=S[:],
                                             start=first, stop=last)
                            nc.tensor.matmul(out=cur_s[:], lhsT=expf[:, t:t + 1], rhs=S[:],
                                             start=first, stop=last)
                            if last:
                                av = acc_v[:, w * 128:(w + 1) * 128]
                                as_ = acc_s[:, w * 128:(w + 1) * 128]
                                if h == 0:
                                    nc.vector.tensor_copy(out=av, in_=cur_v[:])
                                    nc.vector.tensor_copy(out=as_, in_=cur_s[:])
                                else:
                                    nc.vector.tensor_tensor(out=av, in0=av, in1=cur_v[:], op=AL.add)
                                    nc.vector.tensor_tensor(out=as_, in0=as_, in1=cur_s[:], op=AL.add)
                    t_base += Tp

                # --- Phase 4: finalize windows ---
                if 'edge' in skip:
                    nc.vector.memset(acc_v[:], 0.5)
                    nc.vector.memset(acc_s[:], 1.0)
                for w in range(W):
                    cw = min(WD, SH - w * WD)
                    ps_bc = psB.tile([128, 128], f32, tag="pv")
                    nc.tensor.matmul(out=ps_bc[:], lhsT=ones_row[:],
                                     rhs=acc_s[:, w * 128:(w + 1) * 128],
                                     start=True, stop=True)
                    rbc = spool.tile([128, 128], f32, tag="rbc")
                    nc.vector.tensor_scalar(
                        out=rbc[:], in0=ps_bc[:],
                        scalar1=1e-30, scalar2=None, op0=AL.add)
                    nc.vector.reciprocal(rbc[:], rbc[:])
                    onorm = spool.tile([128, 128], f32, tag="onorm")
                    nc.vector.tensor_tensor(
                        out=onorm[:], in0=acc_v[:, w * 128:(w + 1) * 128],
                        in1=rbc[:], op=AL.mult)
                    po = psA.tile([128, 128], f32, tag="pq")
                    nc.tensor.matmul(out=po[:, :cw], lhsT=wo, rhs=onorm[:, :cw],
                                     start=True, stop=True)
                    func = AF.Relu if l < L - 1 else AF.Identity
                    nc.scalar.activation(
                        hnext[:, w * WD:w * WD + cw], po[:, :cw], func,
                        bias=boc_sb[:, l:l + 1])
                hcur, hnext = hnext, hcur

            nc.sync.dma_start(out=outT[:, :], in_=hcur[:])
    nc.compile()
    return nc


def run_spmd(cfg, nc, in_maps):
    from concourse.bass_utils import run_bass_kernel_spmd
    res = run_bass_kernel_spmd(nc, in_maps, list(range(cfg.NC)))
    return res


def run_spmd_trace(cfg, nc, in_maps):
    from concourse.bass_utils import run_bass_kernel_spmd
    return run_bass_kernel_spmd(nc, in_maps, list(range(cfg.NC)), trace=True)


def assemble_output(cfg, results):
    outs = []
    for c in range(cfg.NC):
        outs.append(np.asarray(results[c]['outT']).T)  # [SH, 128]
    return np.ascontiguousarray(np.vstack(outs))


# ----------------------------------------------------------------------------
# Harness entry point: full inputs in, full output out.
# ----------------------------------------------------------------------------
_CACHE = {}


def kernel(**inputs):
    if 'cfg' not in _CACHE:
        _CACHE['cfg'] = Cfg()
        _CACHE['masks'] = compute_masks(_CACHE['cfg'])
    cfg, masks = _CACHE['cfg'], _CACHE['masks']
    counts, meta, Ttot = prep_edges(cfg, inputs['edge_index'], masks)
    key = counts.tobytes()
    if _CACHE.get('counts_key') != key:
        _CACHE['nc'] = build_bass(cfg, counts)
        _CACHE['counts_key'] = key
    in_maps = build_in_maps(cfg, inputs, counts, meta)
    res = run_spmd(cfg, _CACHE['nc'], in_maps)
    return assemble_output(cfg, res.results)


# revision 6
# speedup vs baseline: 1.0493x; 1.0493x over previous
"""BigBird GNN message passing on Trainium2, 8-core SPMD.

Sharding: dst-range sharding (SH dsts per core). Per layer:
  1. Each core computes Q (own dsts, +zero row) and K|V (own node shard)
     from its SBUF-resident h^T slice; K|V packed as KV [SH, 256].
  2. AllGather KV halves -> KV_A/KV_B [NC*HALF, 256] (full graph KV,
     int16-gatherable row spaces < 32768).
  3. Edge phase (2 passes A/B): batched dma_gather of KV rows (by src) and
     Q rows (by dst; uniform/dummy edges hit the zero row so exp(0)=1),
     per 128-edge tile: alpha = rowdot(Qg, Kg)*scale -> exp -> Vg*exp,
     one-hot S = (iota == dstloc), matmul-accumulate S^T-weighted sums
     into per-window PSUM -> SBUF accumulators acc_v [D x dst], acc_s.
  4. Finalize per 127-dst window: r = 1/s, out^T = acc_v * r,
     h'^T = Wo^T @ out^T + bo (+relu), into the next h^T slice.

All-masked dsts are pre-detected on host: their full edge list is included
with Q-index = zero row, reproducing the reference's uniform-average
degeneracy exactly. Masked edges of other dsts contribute exp(-1e9-m)=0
in the reference and are simply dropped here.
"""
import math
import os
from dataclasses import dataclass, field

import numpy as np

# The device path (run_bass_kernel_spmd -> bass2jax PJRT) needs the axon
# platform registered; mask generation explicitly targets the cpu platform.
# Guard against a harness that pins JAX_PLATFORMS=cpu.
_jp = os.environ.get("JAX_PLATFORMS", "")
if "axon" not in _jp:
    os.environ["JAX_PLATFORMS"] = "axon,cpu"


@dataclass
class Cfg:
    N: int = 40000
    E: int = 640000
    D: int = 128
    L: int = 3
    BLOCK: int = 64
    KEEP_P: float = 0.1
    NC: int = 8
    BT: int = 16          # tiles per gather batch
    TILE: int = 128
    WD: int = 127         # dsts per window (col 127 = trash)

    @property
    def SH(self):
        return self.N // self.NC

    @property
    def HALF(self):
        return self.SH // 2

    @property
    def W(self):
        return (self.SH + self.WD - 1) // self.WD

    @property
    def QROWS(self):
        return self.SH + 1

    @property
    def SCALE(self):
        return 1.0 / math.sqrt(self.D)


def compute_masks(cfg):
    """Reference's per-layer random keep masks, bit-exact (CPU jax)."""
    import jax
    out = []
    with jax.default_device(jax.devices('cpu')[0]):
        for i in range(cfg.L):
            rkey = jax.random.fold_in(jax.random.key(42), i)
            out.append(np.asarray(jax.random.uniform(rkey, (cfg.E,)) > cfg.KEEP_P))
    return out


def prep_edges(cfg, edge_index, rand_masks):
    """Per-layer edge metadata. Returns (counts [L,2,W], meta per core, Ttot)."""
    c_ = cfg
    src = np.asarray(edge_index[0]).astype(np.int64)
    dst = np.asarray(edge_index[1]).astype(np.int64)
    local_keep = np.abs(src - dst) <= c_.BLOCK

    counts = np.zeros((c_.L, 2, c_.W), np.int64)
    per = [[[None] * c_.W for _ in range(2 * c_.NC)] for _ in range(c_.L)]

    for l in range(c_.L):
        kept = local_keep | ~rand_masks[l]
        deg_all = np.bincount(dst, minlength=c_.N)
        kept_cnt = np.bincount(dst[kept], minlength=c_.N)
        uniform_dst = (kept_cnt == 0) & (deg_all > 0)
        use = kept | uniform_dst[dst]
        es, ed = src[use], dst[use]
        is_unif = uniform_dst[ed]
        core = ed // c_.SH
        dloc = ed - core * c_.SH
        win = dloc // c_.WD
        srcmod = es % c_.SH
        half = (srcmod >= c_.HALF).astype(np.int64)
        kvrow = c_.HALF * (es // c_.SH) + srcmod - half * c_.HALF
        qrow = np.where(is_unif, c_.SH, dloc)
        dstcol = dloc - win * c_.WD

        order = np.lexsort((es, win, half, core))
        c_o, h_o, w_o = core[order], half[order], win[order]
        kv_o, q_o, dc_o = kvrow[order], qrow[order], dstcol[order]
        key = (c_o * 2 + h_o) * c_.W + w_o
        bounds = np.searchsorted(key, np.arange(c_.NC * 2 * c_.W + 1))
        for c in range(c_.NC):
            for h in range(2):
                for w in range(c_.W):
                    k = (c * 2 + h) * c_.W + w
                    s, e = bounds[k], bounds[k + 1]
                    per[l][c * 2 + h][w] = (kv_o[s:e], q_o[s:e], dc_o[s:e])
                    counts[l, h, w] = max(counts[l, h, w], -(-(e - s) // c_.TILE))
        counts[l, 0, :] = np.maximum(counts[l, 0, :], 1)

    Ttot = int(counts.sum())
    meta = []
    for c in range(c_.NC):
        kv_all = np.zeros((Ttot, c_.TILE), np.int16)
        q_all = np.full((Ttot, c_.TILE), c_.SH, np.int16)
        dl_all = np.full((Ttot, c_.TILE), c_.WD, np.float32)
        t0 = 0
        for l in range(c_.L):
            for h in range(2):
                for w in range(c_.W):
                    kv, q, dc = per[l][c * 2 + h][w]
                    n = len(kv)
                    nt = int(counts[l, h, w])
                    kv_all[t0:t0 + nt].reshape(-1)[:n] = kv
                    q_all[t0:t0 + nt].reshape(-1)[:n] = q
                    dl_all[t0:t0 + nt].reshape(-1)[:n] = dc
                    t0 += nt
        assert t0 == Ttot

        def wrap16(a):
            b = a.reshape(-1).reshape(-1, 16).T  # [16, T*8]
            return np.ascontiguousarray(np.tile(b, (8, 1)).astype(np.int16))
        meta.append({
            'kvix': wrap16(kv_all),
            'qix': wrap16(q_all),
            'dloc': np.ascontiguousarray(dl_all.T),
        })
    return counts, meta, Ttot


def pack_weights(cfg, inp):
    L = cfg.L
    ws = np.zeros((128, L * 4 * 128), np.float32)
    brow = np.zeros((128, L * 3 * 128), np.float32)  # row-bias replicated over partitions
    boc = np.zeros((128, L), np.float32)
    for l in range(L):
        for f, nm in enumerate(['Wq', 'Wk', 'Wv', 'Wo']):
            ws[:, (l * 4 + f) * 128:(l * 4 + f + 1) * 128] = np.asarray(inp[nm][l], np.float32)
        for f, nm in enumerate(['bq', 'bk', 'bv']):
            brow[:, (l * 3 + f) * 128:(l * 3 + f + 1) * 128] = np.asarray(inp[nm][l], np.float32)[None, :]
        boc[:, l] = np.asarray(inp['bo'][l], np.float32)
    return ws, brow, boc


def build_in_maps(cfg, inputs, counts, meta):
    x = np.asarray(inputs['x'], np.float32)
    ws, brow, boc = pack_weights(cfg, inputs)
    iota = np.broadcast_to(np.arange(128, dtype=np.float32)[None, :], (128, 128)).copy()
    in_maps = []
    for c in range(cfg.NC):
        xT = np.ascontiguousarray(x[c * cfg.SH:(c + 1) * cfg.SH].T)
        in_maps.append({
            'xT': xT, 'ws': ws, 'brow': brow, 'boc': boc, 'iota': iota,
            'kvix': meta[c]['kvix'], 'qix': meta[c]['qix'], 'dloc': meta[c]['dloc'],
        })
    return in_maps


def build_bass(cfg, counts, skip=()):
    import os
    import concourse.bacc as bacc
    import concourse.mybir as mybir
    import concourse.tile as tile
    from concourse import library_config

    c_ = cfg
    f32 = mybir.dt.float32
    i16 = mybir.dt.int16
    AL = mybir.AluOpType
    AF = mybir.ActivationFunctionType
    Ttot = int(counts.sum())
    SH, HALF, W, WD, BT, L, D = c_.SH, c_.HALF, c_.W, c_.WD, c_.BT, c_.L, c_.D
    NCH = -(-SH // 128)           # node chunks for QKV phase

    nc = bacc.Bacc("TRN2", num_devices=c_.NC)

    xT = nc.dram_tensor("xT", [128, SH], f32, kind="ExternalInput")
    ws = nc.dram_tensor("ws", [128, L * 4 * 128], f32, kind="ExternalInput")
    brow = nc.dram_tensor("brow", [128, L * 3 * 128], f32, kind="ExternalInput")
    boc = nc.dram_tensor("boc", [128, L], f32, kind="ExternalInput")
    iota = nc.dram_tensor("iota", [128, 128], f32, kind="ExternalInput")
    kvix = nc.dram_tensor("kvix", [128, Ttot * 8], i16, kind="ExternalInput")
    qix = nc.dram_tensor("qix", [128, Ttot * 8], i16, kind="ExternalInput")
    dloc = nc.dram_tensor("dloc", [128, Ttot], f32, kind="ExternalInput")
    outT = nc.dram_tensor("outT", [128, SH], f32, kind="ExternalOutput")

    Qd = nc.dram_tensor("Qd", [c_.QROWS, 128], f32)
    KVown = nc.dram_tensor("KVown", [SH, 256], f32)
    kv_space = "Local" if 'coll' in skip else "Shared"
    KV_A = nc.dram_tensor("KV_A", [c_.NC * HALF, 256], f32, addr_space=kv_space)
    KV_B = nc.dram_tensor("KV_B", [c_.NC * HALF, 256], f32, addr_space=kv_space)
    groups = [list(range(c_.NC))]

    with tile.TileContext(nc) as tc:
        with (
            tc.tile_pool(name="const", bufs=1) as cpool,
            tc.tile_pool(name="hbuf", bufs=1) as hpool,
            tc.tile_pool(name="acc", bufs=1) as apool,
            tc.tile_pool(name="qkv", bufs=3) as qkvpool,
            tc.tile_pool(name="gath", bufs=2) as gpool,
            tc.tile_pool(name="scr", bufs=4) as spool,
            tc.tile_pool(name="psA", bufs=2, space="PSUM") as psA,
            tc.tile_pool(name="psB", bufs=2, space="PSUM") as psB,
        ):
            nc.gpsimd.load_library(library_config.mlp)
            nidx_regs = {}  # num_idxs value -> Pool register (reused)

            def nidx_reg(v):
                if v not in nidx_regs:
                    nidx_regs[v] = nc.gpsimd.to_reg(v)
                return nidx_regs[v]
            iota_sb = cpool.tile([128, 128], f32, tag="iota")
            ws_sb = cpool.tile([128, L * 4 * 128], f32, tag="ws")
            brow_sb = cpool.tile([128, L * 3 * 128], f32, tag="brow")
            ones_row = cpool.tile([1, 128], f32, tag="ones")
            boc_sb = cpool.tile([128, L], f32, tag="boc")
            zrow = cpool.tile([1, 128], f32, tag="zrow")
            nc.sync.dma_start(out=iota_sb[:], in_=iota[:, :])
            nc.sync.dma_start(out=ws_sb[:], in_=ws[:, :])
            nc.sync.dma_start(out=brow_sb[:], in_=brow[:, :])
            nc.sync.dma_start(out=boc_sb[:], in_=boc[:, :])
            nc.vector.memset(zrow[:], 0.0)
            nc.vector.memset(ones_row[:], 1.0)
            nc.sync.dma_start(out=Qd[SH:SH + 1, :], in_=zrow[:])

            hA = hpool.tile([128, SH], f32, tag="hA")
            hB = hpool.tile([128, SH], f32, tag="hB")
            acc_v = apool.tile([128, W * 128], f32, tag="accv")
            acc_s = apool.tile([1, W * 128], f32, tag="accs")
            nc.sync.dma_start(out=hA[:], in_=xT[:, :])

            hcur, hnext = hA, hB
            t_base = 0
            for l in range(L):
                wq = ws_sb[:, (l * 4 + 0) * 128:(l * 4 + 1) * 128]
                wk = ws_sb[:, (l * 4 + 1) * 128:(l * 4 + 2) * 128]
                wv = ws_sb[:, (l * 4 + 2) * 128:(l * 4 + 3) * 128]
                wo = ws_sb[:, (l * 4 + 3) * 128:(l * 4 + 4) * 128]
                bq = brow_sb[:, (l * 3 + 0) * 128:(l * 3 + 1) * 128]
                bk = brow_sb[:, (l * 3 + 1) * 128:(l * 3 + 2) * 128]
                bv = brow_sb[:, (l * 3 + 2) * 128:(l * 3 + 3) * 128]

                # --- Phase 1: Q / KV for own shard ---
                for ch in range(NCH):
                    cw = min(128, SH - ch * 128)
                    hs = hcur[:, ch * 128:ch * 128 + cw]
                    pq = psA.tile([128, 128], f32, tag="pq")
                    pkv = psA.tile([128, 256], f32, tag="pkv")
                    nc.tensor.matmul(out=pq[:cw, :], lhsT=hs, rhs=wq, start=True, stop=True)
                    nc.tensor.matmul(out=pkv[:cw, 0:128], lhsT=hs, rhs=wk, start=True, stop=True)
                    nc.tensor.matmul(out=pkv[:cw, 128:256], lhsT=hs, rhs=wv, start=True, stop=True)
                    qt = qkvpool.tile([128, 128], f32, tag="qt")
                    kvt = qkvpool.tile([128, 256], f32, tag="kvt")
                    nc.vector.tensor_tensor(
                        out=qt[:cw, :], in0=pq[:cw, :],
                        in1=bq[:cw, :], op=AL.add)
                    nc.vector.tensor_tensor(
                        out=kvt[:cw, 0:128], in0=pkv[:cw, 0:128],
                        in1=bk[:cw, :], op=AL.add)
                    nc.vector.tensor_tensor(
                        out=kvt[:cw, 128:256], in0=pkv[:cw, 128:256],
                        in1=bv[:cw, :], op=AL.add)
                    nc.sync.dma_start(out=Qd[ch * 128:ch * 128 + cw, :], in_=qt[:cw, :])
                    nc.sync.dma_start(out=KVown[ch * 128:ch * 128 + cw, :], in_=kvt[:cw, :])

                # --- Phase 2+3: AllGather KV halves interleaved with edge
                # passes: AG(half B) is issued after pass A's work so its
                # transfer overlaps pass-A compute.
                if 'coll' not in skip:
                    nc.gpsimd.collective_compute(
                        "AllGather", AL.bypass, replica_groups=groups,
                        ins=[KVown[0:HALF, :]], outs=[KV_A[:, :]])
                for h in range(2 if 'edge' not in skip else 0):
                    if h == 1 and 'coll' not in skip:
                        pass  # AG(B) already issued at end of pass A
                    kvsrc = KV_A if h == 0 else KV_B
                    # tile schedule: (window, first, last) per tile of this pass
                    sched = []
                    for w in range(W):
                        nt = int(counts[l, h, w])
                        for t in range(nt):
                            sched.append((w, t == 0, t == nt - 1))
                    Tp = len(sched)
                    cur_v, cur_s = None, None
                    for b0 in range(0, Tp, BT):
                        bt = min(BT, Tp - b0)
                        g0 = t_base + b0
                        kvi_sb = gpool.tile([128, BT * 8], i16, tag="kvi")
                        qi_sb = gpool.tile([128, BT * 8], i16, tag="qi")
                        dl_sb = gpool.tile([128, BT], f32, tag="dl")
                        nc.sync.dma_start(out=kvi_sb[:, :bt * 8], in_=kvix[:, g0 * 8:(g0 + bt) * 8])
                        nc.sync.dma_start(out=qi_sb[:, :bt * 8], in_=qix[:, g0 * 8:(g0 + bt) * 8])
                        nc.sync.dma_start(out=dl_sb[:, :bt], in_=dloc[:, g0:g0 + bt])
                        kvg = gpool.tile([128, BT, 256], f32, tag="kvg")
                        qg = gpool.tile([128, BT, 128], f32, tag="qg")
                        nc.gpsimd.dma_gather(
                            out_ap=kvg[:, :bt, :], in_ap=kvsrc[:, :],
                            idxs_ap=kvi_sb[:, :bt * 8],
                            num_idxs=bt * 128, num_idxs_reg=nidx_reg(bt * 128),
                            elem_size=256, single_packet=False)
                        nc.gpsimd.dma_gather(
                            out_ap=qg[:, :bt, :], in_ap=Qd[:, :],
                            idxs_ap=qi_sb[:, :bt * 8],
                            num_idxs=bt * 128, num_idxs_reg=nidx_reg(bt * 128),
                            elem_size=128, single_packet=False)
                        prod = gpool.tile([128, BT, 128], f32, tag="prod")
                        nc.vector.tensor_tensor(
                            out=prod[:, :bt, :], in0=qg[:, :bt, :],
                            in1=kvg[:, :bt, 0:128], op=AL.mult)
                        alpha = spool.tile([128, BT], f32, tag="alpha")
                        nc.vector.tensor_reduce(
                            out=alpha[:, :bt, None], in_=prod[:, :bt, :],
                            axis=mybir.AxisListType.X, op=AL.add)
                        expf = spool.tile([128, BT], f32, tag="expf")
                        nc.scalar.activation(expf[:, :bt], alpha[:, :bt], AF.Exp,
                                             scale=float(c_.SCALE))
                        vex = gpool.tile([128, BT, 128], f32, tag="vex")
                        nc.vector.tensor_tensor(
                            out=vex[:, :bt, :], in0=kvg[:, :bt, 128:256],
                            in1=expf[:, :bt, None].to_broadcast([128, bt, 128]),
                            op=AL.mult)
                        for t in range(bt):
                            w, first, last = sched[b0 + t]
                            S = spool.tile([128, 128], f32, tag="S")
                            nc.vector.tensor_scalar(
                                out=S[:], in0=iota_sb[:],
                                scalar1=dl_sb[:, t:t + 1], scalar2=None, op0=AL.is_equal)
                            if first:
                                cur_v = psB.tile([128, 128], f32, tag="pv")
                                cur_s = psB.tile([1, 128], f32, tag="ps")
                            nc.tensor.matmul(out=cur_v[:], lhsT=vex[:, t, :], rhs=S[:],
                                             start=first, stop=last)
                            nc.tensor.matmul(out=cur_s[:], lhsT=expf[:, t:t + 1], rhs=S[:],
                                             start=first, stop=last)
                            if last:
                                av = acc_v[:, w * 128:(w + 1) * 128]
                                as_ = acc_s[:, w * 128:(w + 1) * 128]
                                if h == 0:
                                    nc.vector.tensor_copy(out=av, in_=cur_v[:])
                                    nc.vector.tensor_copy(out=as_, in_=cur_s[:])
                                else:
                                    nc.vector.tensor_tensor(out=av, in0=av, in1=cur_v[:], op=AL.add)
                                    nc.vector.tensor_tensor(out=as_, in0=as_, in1=cur_s[:], op=AL.add)
                    t_base += Tp
                    if h == 0 and 'coll' not in skip:
                        nc.gpsimd.collective_compute(
                            "AllGather", AL.bypass, replica_groups=groups,
                            ins=[KVown[HALF:SH, :]], outs=[KV_B[:, :]])

                # --- Phase 4: finalize windows ---
                if 'edge' in skip:
                    nc.vector.memset(acc_v[:], 0.5)
                    nc.vector.memset(acc_s[:], 1.0)
                for w in range(W):
                    cw = min(WD, SH - w * WD)
                    ps_bc = psB.tile([128, 128], f32, tag="pv")
                    nc.tensor.matmul(out=ps_bc[:], lhsT=ones_row[:],
                                     rhs=acc_s[:, w * 128:(w + 1) * 128],
                                     start=True, stop=True)
                    rbc = spool.tile([128, 128], f32, tag="rbc")
                    nc.vector.tensor_scalar(
                        out=rbc[:], in0=ps_bc[:],
                        scalar1=1e-30, scalar2=None, op0=AL.add)
                    nc.vector.reciprocal(rbc[:], rbc[:])
                    onorm = spool.tile([128, 128], f32, tag="onorm")
                    nc.vector.tensor_tensor(
                        out=onorm[:], in0=acc_v[:, w * 128:(w + 1) * 128],
                        in1=rbc[:], op=AL.mult)
                    po = psA.tile([128, 128], f32, tag="pq")
                    nc.tensor.matmul(out=po[:, :cw], lhsT=wo, rhs=onorm[:, :cw],
                                     start=True, stop=True)
                    func = AF.Relu if l < L - 1 else AF.Identity
                    nc.scalar.activation(
                        hnext[:, w * WD:w * WD + cw], po[:, :cw], func,
                        bias=boc_sb[:, l:l + 1])
                hcur, hnext = hnext, hcur

            nc.sync.dma_start(out=outT[:, :], in_=hcur[:])
    nc.compile()
    return nc


def run_spmd(cfg, nc, in_maps):
    from concourse.bass_utils import run_bass_kernel_spmd
    res = run_bass_kernel_spmd(nc, in_maps, list(range(cfg.NC)))
    return res


def run_spmd_trace(cfg, nc, in_maps):
    from concourse.bass_utils import run_bass_kernel_spmd
    return run_bass_kernel_spmd(nc, in_maps, list(range(cfg.NC)), trace=True)


def assemble_output(cfg, results):
    outs = []
    for c in range(cfg.NC):
        outs.append(np.asarray(results[c]['outT']).T)  # [SH, 128]
    return np.ascontiguousarray(np.vstack(outs))


# ----------------------------------------------------------------------------
# Harness entry point: full inputs in, full output out.
# ----------------------------------------------------------------------------
_CACHE = {}


def kernel(**inputs):
    if 'cfg' not in _CACHE:
        _CACHE['cfg'] = Cfg()
        _CACHE['masks'] = compute_masks(_CACHE['cfg'])
    cfg, masks = _CACHE['cfg'], _CACHE['masks']
    counts, meta, Ttot = prep_edges(cfg, inputs['edge_index'], masks)
    key = counts.tobytes()
    if _CACHE.get('counts_key') != key:
        _CACHE['nc'] = build_bass(cfg, counts)
        _CACHE['counts_key'] = key
    in_maps = build_in_maps(cfg, inputs, counts, meta)
    res = run_spmd(cfg, _CACHE['nc'], in_maps)
    return assemble_output(cfg, res.results)


# revision 7
# speedup vs baseline: 1.0533x; 1.0038x over previous
"""BigBird GNN message passing on Trainium2, 8-core SPMD.

Sharding: dst-range sharding (SH dsts per core). Per layer:
  1. Each core computes Q (own dsts, +zero row) and K|V (own node shard)
     from its SBUF-resident h^T slice; K|V packed as KV [SH, 256].
  2. AllGather KV halves -> KV_A/KV_B [NC*HALF, 256] (full graph KV,
     int16-gatherable row spaces < 32768).
  3. Edge phase (2 passes A/B): batched dma_gather of KV rows (by src) and
     Q rows (by dst; uniform/dummy edges hit the zero row so exp(0)=1),
     per 128-edge tile: alpha = rowdot(Qg, Kg)*scale -> exp -> Vg*exp,
     one-hot S = (iota == dstloc), matmul-accumulate S^T-weighted sums
     into per-window PSUM -> SBUF accumulators acc_v [D x dst], acc_s.
  4. Finalize per 127-dst window: r = 1/s, out^T = acc_v * r,
     h'^T = Wo^T @ out^T + bo (+relu), into the next h^T slice.

All-masked dsts are pre-detected on host: their full edge list is included
with Q-index = zero row, reproducing the reference's uniform-average
degeneracy exactly. Masked edges of other dsts contribute exp(-1e9-m)=0
in the reference and are simply dropped here.
"""
import math
import os
from dataclasses import dataclass, field

import numpy as np

# The device path (run_bass_kernel_spmd -> bass2jax PJRT) needs the axon
# platform registered; mask generation explicitly targets the cpu platform.
# Guard against a harness that pins JAX_PLATFORMS=cpu.
_jp = os.environ.get("JAX_PLATFORMS", "")
if "axon" not in _jp:
    os.environ["JAX_PLATFORMS"] = "axon,cpu"


@dataclass
class Cfg:
    N: int = 40000
    E: int = 640000
    D: int = 128
    L: int = 3
    BLOCK: int = 64
    KEEP_P: float = 0.1
    NC: int = 8
    BT: int = 16          # tiles per gather batch
    TILE: int = 128
    WD: int = 127         # dsts per window (col 127 = trash)

    @property
    def SH(self):
        return self.N // self.NC

    @property
    def HALF(self):
        return self.SH // 2

    @property
    def W(self):
        return (self.SH + self.WD - 1) // self.WD

    @property
    def QROWS(self):
        return self.SH + 1

    @property
    def SCALE(self):
        return 1.0 / math.sqrt(self.D)


def compute_masks(cfg):
    """Reference's per-layer random keep masks, bit-exact (CPU jax)."""
    import jax
    out = []
    with jax.default_device(jax.devices('cpu')[0]):
        for i in range(cfg.L):
            rkey = jax.random.fold_in(jax.random.key(42), i)
            out.append(np.asarray(jax.random.uniform(rkey, (cfg.E,)) > cfg.KEEP_P))
    return out


def prep_edges(cfg, edge_index, rand_masks):
    """Per-layer edge metadata. Returns (counts [L,2,W], meta per core, Ttot)."""
    c_ = cfg
    src = np.asarray(edge_index[0]).astype(np.int64)
    dst = np.asarray(edge_index[1]).astype(np.int64)
    local_keep = np.abs(src - dst) <= c_.BLOCK

    counts = np.zeros((c_.L, 2, c_.W), np.int64)
    per = [[[None] * c_.W for _ in range(2 * c_.NC)] for _ in range(c_.L)]

    for l in range(c_.L):
        kept = local_keep | ~rand_masks[l]
        deg_all = np.bincount(dst, minlength=c_.N)
        kept_cnt = np.bincount(dst[kept], minlength=c_.N)
        uniform_dst = (kept_cnt == 0) & (deg_all > 0)
        use = kept | uniform_dst[dst]
        es, ed = src[use], dst[use]
        is_unif = uniform_dst[ed]
        core = ed // c_.SH
        dloc = ed - core * c_.SH
        win = dloc // c_.WD
        half = (es >= c_.N // 2).astype(np.int64)
        kvrow = es - half * (c_.N // 2)
        qrow = np.where(is_unif, c_.SH, dloc)
        dstcol = dloc - win * c_.WD

        order = np.lexsort((es, win, half, core))
        c_o, h_o, w_o = core[order], half[order], win[order]
        kv_o, q_o, dc_o = kvrow[order], qrow[order], dstcol[order]
        key = (c_o * 2 + h_o) * c_.W + w_o
        bounds = np.searchsorted(key, np.arange(c_.NC * 2 * c_.W + 1))
        for c in range(c_.NC):
            for h in range(2):
                for w in range(c_.W):
                    k = (c * 2 + h) * c_.W + w
                    s, e = bounds[k], bounds[k + 1]
                    per[l][c * 2 + h][w] = (kv_o[s:e], q_o[s:e], dc_o[s:e])
                    counts[l, h, w] = max(counts[l, h, w], -(-(e - s) // c_.TILE))
        counts[l, 0, :] = np.maximum(counts[l, 0, :], 1)

    Ttot = int(counts.sum())
    meta = []
    for c in range(c_.NC):
        kv_all = np.zeros((Ttot, c_.TILE), np.int16)
        q_all = np.full((Ttot, c_.TILE), c_.SH, np.int16)
        dl_all = np.full((Ttot, c_.TILE), c_.WD, np.float32)
        t0 = 0
        for l in range(c_.L):
            for h in range(2):
                for w in range(c_.W):
                    kv, q, dc = per[l][c * 2 + h][w]
                    n = len(kv)
                    nt = int(counts[l, h, w])
                    kv_all[t0:t0 + nt].reshape(-1)[:n] = kv
                    q_all[t0:t0 + nt].reshape(-1)[:n] = q
                    dl_all[t0:t0 + nt].reshape(-1)[:n] = dc
                    t0 += nt
        assert t0 == Ttot

        def wrap16(a):
            b = a.reshape(-1).reshape(-1, 16).T  # [16, T*8]
            return np.ascontiguousarray(np.tile(b, (8, 1)).astype(np.int16))
        meta.append({
            'kvix': wrap16(kv_all),
            'qix': wrap16(q_all),
            'dloc': np.ascontiguousarray(dl_all.T),
        })
    return counts, meta, Ttot


def pack_weights(cfg, inp):
    L = cfg.L
    ws = np.zeros((128, L * 4 * 128), np.float32)
    brow = np.zeros((128, L * 3 * 128), np.float32)  # row-bias replicated over partitions
    boc = np.zeros((128, L), np.float32)
    for l in range(L):
        for f, nm in enumerate(['Wq', 'Wk', 'Wv', 'Wo']):
            ws[:, (l * 4 + f) * 128:(l * 4 + f + 1) * 128] = np.asarray(inp[nm][l], np.float32)
        for f, nm in enumerate(['bq', 'bk', 'bv']):
            brow[:, (l * 3 + f) * 128:(l * 3 + f + 1) * 128] = np.asarray(inp[nm][l], np.float32)[None, :]
        boc[:, l] = np.asarray(inp['bo'][l], np.float32)
    return ws, brow, boc


def build_in_maps(cfg, inputs, counts, meta):
    x = np.asarray(inputs['x'], np.float32)
    ws, brow, boc = pack_weights(cfg, inputs)
    iota = np.broadcast_to(np.arange(128, dtype=np.float32)[None, :], (128, 128)).copy()
    in_maps = []
    for c in range(cfg.NC):
        xT = np.ascontiguousarray(x[c * cfg.SH:(c + 1) * cfg.SH].T)
        in_maps.append({
            'xT': xT, 'ws': ws, 'brow': brow, 'boc': boc, 'iota': iota,
            'kvix': meta[c]['kvix'], 'qix': meta[c]['qix'], 'dloc': meta[c]['dloc'],
        })
    return in_maps


def build_bass(cfg, counts, skip=()):
    import os
    import concourse.bacc as bacc
    import concourse.mybir as mybir
    import concourse.tile as tile
    from concourse import library_config

    c_ = cfg
    f32 = mybir.dt.float32
    i16 = mybir.dt.int16
    AL = mybir.AluOpType
    AF = mybir.ActivationFunctionType
    Ttot = int(counts.sum())
    SH, HALF, W, WD, BT, L, D = c_.SH, c_.HALF, c_.W, c_.WD, c_.BT, c_.L, c_.D
    NCH = -(-SH // 128)           # node chunks for QKV phase

    nc = bacc.Bacc("TRN2", num_devices=c_.NC)

    xT = nc.dram_tensor("xT", [128, SH], f32, kind="ExternalInput")
    ws = nc.dram_tensor("ws", [128, L * 4 * 128], f32, kind="ExternalInput")
    brow = nc.dram_tensor("brow", [128, L * 3 * 128], f32, kind="ExternalInput")
    boc = nc.dram_tensor("boc", [128, L], f32, kind="ExternalInput")
    iota = nc.dram_tensor("iota", [128, 128], f32, kind="ExternalInput")
    kvix = nc.dram_tensor("kvix", [128, Ttot * 8], i16, kind="ExternalInput")
    qix = nc.dram_tensor("qix", [128, Ttot * 8], i16, kind="ExternalInput")
    dloc = nc.dram_tensor("dloc", [128, Ttot], f32, kind="ExternalInput")
    outT = nc.dram_tensor("outT", [128, SH], f32, kind="ExternalOutput")

    Qd = nc.dram_tensor("Qd", [c_.QROWS, 128], f32)
    KVown = nc.dram_tensor("KVown", [SH, 256], f32)
    kv_space = "Local" if 'coll' in skip else "Shared"
    KV_F = nc.dram_tensor("KV_F", [c_.N, 256], f32, addr_space=kv_space)
    groups = [list(range(c_.NC))]

    with tile.TileContext(nc) as tc:
        with (
            tc.tile_pool(name="const", bufs=1) as cpool,
            tc.tile_pool(name="hbuf", bufs=1) as hpool,
            tc.tile_pool(name="acc", bufs=1) as apool,
            tc.tile_pool(name="qkv", bufs=3) as qkvpool,
            tc.tile_pool(name="gath", bufs=2) as gpool,
            tc.tile_pool(name="scr", bufs=4) as spool,
            tc.tile_pool(name="psA", bufs=2, space="PSUM") as psA,
            tc.tile_pool(name="psB", bufs=2, space="PSUM") as psB,
        ):
            nc.gpsimd.load_library(library_config.mlp)
            nidx_regs = {}  # num_idxs value -> Pool register (reused)

            def nidx_reg(v):
                if v not in nidx_regs:
                    nidx_regs[v] = nc.gpsimd.to_reg(v)
                return nidx_regs[v]
            iota_sb = cpool.tile([128, 128], f32, tag="iota")
            ws_sb = cpool.tile([128, L * 4 * 128], f32, tag="ws")
            brow_sb = cpool.tile([128, L * 3 * 128], f32, tag="brow")
            ones_row = cpool.tile([1, 128], f32, tag="ones")
            boc_sb = cpool.tile([128, L], f32, tag="boc")
            zrow = cpool.tile([1, 128], f32, tag="zrow")
            nc.sync.dma_start(out=iota_sb[:], in_=iota[:, :])
            nc.sync.dma_start(out=ws_sb[:], in_=ws[:, :])
            nc.sync.dma_start(out=brow_sb[:], in_=brow[:, :])
            nc.sync.dma_start(out=boc_sb[:], in_=boc[:, :])
            nc.vector.memset(zrow[:], 0.0)
            nc.vector.memset(ones_row[:], 1.0)
            nc.sync.dma_start(out=Qd[SH:SH + 1, :], in_=zrow[:])

            hA = hpool.tile([128, SH], f32, tag="hA")
            hB = hpool.tile([128, SH], f32, tag="hB")
            acc_v = apool.tile([128, W * 128], f32, tag="accv")
            acc_s = apool.tile([1, W * 128], f32, tag="accs")
            nc.sync.dma_start(out=hA[:], in_=xT[:, :])

            hcur, hnext = hA, hB
            t_base = 0
            for l in range(L):
                wq = ws_sb[:, (l * 4 + 0) * 128:(l * 4 + 1) * 128]
                wk = ws_sb[:, (l * 4 + 1) * 128:(l * 4 + 2) * 128]
                wv = ws_sb[:, (l * 4 + 2) * 128:(l * 4 + 3) * 128]
                wo = ws_sb[:, (l * 4 + 3) * 128:(l * 4 + 4) * 128]
                bq = brow_sb[:, (l * 3 + 0) * 128:(l * 3 + 1) * 128]
                bk = brow_sb[:, (l * 3 + 1) * 128:(l * 3 + 2) * 128]
                bv = brow_sb[:, (l * 3 + 2) * 128:(l * 3 + 3) * 128]

                # --- Phase 1: Q / KV for own shard ---
                for ch in range(NCH):
                    cw = min(128, SH - ch * 128)
                    hs = hcur[:, ch * 128:ch * 128 + cw]
                    pq = psA.tile([128, 128], f32, tag="pq")
                    pkv = psA.tile([128, 256], f32, tag="pkv")
                    nc.tensor.matmul(out=pq[:cw, :], lhsT=hs, rhs=wq, start=True, stop=True)
                    nc.tensor.matmul(out=pkv[:cw, 0:128], lhsT=hs, rhs=wk, start=True, stop=True)
                    nc.tensor.matmul(out=pkv[:cw, 128:256], lhsT=hs, rhs=wv, start=True, stop=True)
                    qt = qkvpool.tile([128, 128], f32, tag="qt")
                    kvt = qkvpool.tile([128, 256], f32, tag="kvt")
                    nc.vector.tensor_tensor(
                        out=qt[:cw, :], in0=pq[:cw, :],
                        in1=bq[:cw, :], op=AL.add)
                    nc.vector.tensor_tensor(
                        out=kvt[:cw, 0:128], in0=pkv[:cw, 0:128],
                        in1=bk[:cw, :], op=AL.add)
                    nc.vector.tensor_tensor(
                        out=kvt[:cw, 128:256], in0=pkv[:cw, 128:256],
                        in1=bv[:cw, :], op=AL.add)
                    nc.sync.dma_start(out=Qd[ch * 128:ch * 128 + cw, :], in_=qt[:cw, :])
                    nc.sync.dma_start(out=KVown[ch * 128:ch * 128 + cw, :], in_=kvt[:cw, :])

                # --- Phase 2: single AllGather of the full KV shard ---
                if 'coll' not in skip:
                    nc.gpsimd.collective_compute(
                        "AllGather", AL.bypass, replica_groups=groups,
                        ins=[KVown[:, :]], outs=[KV_F[:, :]])
                for h in range(2 if 'edge' not in skip else 0):
                    kvsrc = KV_F[0:c_.N // 2, :] if h == 0 else KV_F[c_.N // 2:c_.N, :]
                    # tile schedule: (window, first, last) per tile of this pass
                    sched = []
                    for w in range(W):
                        nt = int(counts[l, h, w])
                        for t in range(nt):
                            sched.append((w, t == 0, t == nt - 1))
                    Tp = len(sched)
                    cur_v, cur_s = None, None
                    for b0 in range(0, Tp, BT):
                        bt = min(BT, Tp - b0)
                        g0 = t_base + b0
                        kvi_sb = gpool.tile([128, BT * 8], i16, tag="kvi")
                        qi_sb = gpool.tile([128, BT * 8], i16, tag="qi")
                        dl_sb = gpool.tile([128, BT], f32, tag="dl")
                        nc.sync.dma_start(out=kvi_sb[:, :bt * 8], in_=kvix[:, g0 * 8:(g0 + bt) * 8])
                        nc.sync.dma_start(out=qi_sb[:, :bt * 8], in_=qix[:, g0 * 8:(g0 + bt) * 8])
                        nc.sync.dma_start(out=dl_sb[:, :bt], in_=dloc[:, g0:g0 + bt])
                        kvg = gpool.tile([128, BT, 256], f32, tag="kvg")
                        qg = gpool.tile([128, BT, 128], f32, tag="qg")
                        nc.gpsimd.dma_gather(
                            out_ap=kvg[:, :bt, :], in_ap=kvsrc,
                            idxs_ap=kvi_sb[:, :bt * 8],
                            num_idxs=bt * 128, num_idxs_reg=nidx_reg(bt * 128),
                            elem_size=256, single_packet=False)
                        nc.gpsimd.dma_gather(
                            out_ap=qg[:, :bt, :], in_ap=Qd[:, :],
                            idxs_ap=qi_sb[:, :bt * 8],
                            num_idxs=bt * 128, num_idxs_reg=nidx_reg(bt * 128),
                            elem_size=128, single_packet=False)
                        prod = gpool.tile([128, BT, 128], f32, tag="prod")
                        nc.vector.tensor_tensor(
                            out=prod[:, :bt, :], in0=qg[:, :bt, :],
                            in1=kvg[:, :bt, 0:128], op=AL.mult)
                        alpha = spool.tile([128, BT], f32, tag="alpha")
                        nc.vector.tensor_reduce(
                            out=alpha[:, :bt, None], in_=prod[:, :bt, :],
                            axis=mybir.AxisListType.X, op=AL.add)
                        expf = spool.tile([128, BT], f32, tag="expf")
                        nc.scalar.activation(expf[:, :bt], alpha[:, :bt], AF.Exp,
                                             scale=float(c_.SCALE))
                        vex = gpool.tile([128, BT, 128], f32, tag="vex")
                        nc.vector.tensor_tensor(
                            out=vex[:, :bt, :], in0=kvg[:, :bt, 128:256],
                            in1=expf[:, :bt, None].to_broadcast([128, bt, 128]),
                            op=AL.mult)
                        for t in range(bt):
                            w, first, last = sched[b0 + t]
                            S = spool.tile([128, 128], f32, tag="S")
                            nc.vector.tensor_scalar(
                                out=S[:], in0=iota_sb[:],
                                scalar1=dl_sb[:, t:t + 1], scalar2=None, op0=AL.is_equal)
                            if first:
                                cur_v = psB.tile([128, 128], f32, tag="pv")
                                cur_s = psB.tile([1, 128], f32, tag="ps")
                            nc.tensor.matmul(out=cur_v[:], lhsT=vex[:, t, :], rhs=S[:],
                                             start=first, stop=last)
                            nc.tensor.matmul(out=cur_s[:], lhsT=expf[:, t:t + 1], rhs=S[:],
                                             start=first, stop=last)
                            if last:
                                av = acc_v[:, w * 128:(w + 1) * 128]
                                as_ = acc_s[:, w * 128:(w + 1) * 128]
                                if h == 0:
                                    nc.vector.tensor_copy(out=av, in_=cur_v[:])
                                    nc.vector.tensor_copy(out=as_, in_=cur_s[:])
                                else:
                                    nc.vector.tensor_tensor(out=av, in0=av, in1=cur_v[:], op=AL.add)
                                    nc.vector.tensor_tensor(out=as_, in0=as_, in1=cur_s[:], op=AL.add)
                    t_base += Tp

                # --- Phase 4: finalize windows ---
                if 'edge' in skip:
                    nc.vector.memset(acc_v[:], 0.5)
                    nc.vector.memset(acc_s[:], 1.0)
                for w in range(W):
                    cw = min(WD, SH - w * WD)
                    ps_bc = psB.tile([128, 128], f32, tag="pv")
                    nc.tensor.matmul(out=ps_bc[:], lhsT=ones_row[:],
                                     rhs=acc_s[:, w * 128:(w + 1) * 128],
                                     start=True, stop=True)
                    rbc = spool.tile([128, 128], f32, tag="rbc")
                    nc.vector.tensor_scalar(
                        out=rbc[:], in0=ps_bc[:],
                        scalar1=1e-30, scalar2=None, op0=AL.add)
                    nc.vector.reciprocal(rbc[:], rbc[:])
                    onorm = spool.tile([128, 128], f32, tag="onorm")
                    nc.vector.tensor_tensor(
                        out=onorm[:], in0=acc_v[:, w * 128:(w + 1) * 128],
                        in1=rbc[:], op=AL.mult)
                    po = psA.tile([128, 128], f32, tag="pq")
                    nc.tensor.matmul(out=po[:, :cw], lhsT=wo, rhs=onorm[:, :cw],
                                     start=True, stop=True)
                    func = AF.Relu if l < L - 1 else AF.Identity
                    nc.scalar.activation(
                        hnext[:, w * WD:w * WD + cw], po[:, :cw], func,
                        bias=boc_sb[:, l:l + 1])
                hcur, hnext = hnext, hcur

            nc.sync.dma_start(out=outT[:, :], in_=hcur[:])
    nc.compile()
    return nc


def run_spmd(cfg, nc, in_maps):
    from concourse.bass_utils import run_bass_kernel_spmd
    res = run_bass_kernel_spmd(nc, in_maps, list(range(cfg.NC)))
    return res


def run_spmd_trace(cfg, nc, in_maps):
    from concourse.bass_utils import run_bass_kernel_spmd
    return run_bass_kernel_spmd(nc, in_maps, list(range(cfg.NC)), trace=True)


def assemble_output(cfg, results):
    outs = []
    for c in range(cfg.NC):
        outs.append(np.asarray(results[c]['outT']).T)  # [SH, 128]
    return np.ascontiguousarray(np.vstack(outs))


# ----------------------------------------------------------------------------
# Harness entry point: full inputs in, full output out.
# ----------------------------------------------------------------------------
_CACHE = {}


def kernel(**inputs):
    if 'cfg' not in _CACHE:
        _CACHE['cfg'] = Cfg()
        _CACHE['masks'] = compute_masks(_CACHE['cfg'])
    cfg, masks = _CACHE['cfg'], _CACHE['masks']
    counts, meta, Ttot = prep_edges(cfg, inputs['edge_index'], masks)
    key = counts.tobytes()
    if _CACHE.get('counts_key') != key:
        _CACHE['nc'] = build_bass(cfg, counts)
        _CACHE['counts_key'] = key
    in_maps = build_in_maps(cfg, inputs, counts, meta)
    res = run_spmd(cfg, _CACHE['nc'], in_maps)
    return assemble_output(cfg, res.results)
